# revision 16
# baseline (speedup 1.0000x reference)
"""AttLIF Trainium2 kernel (8-core data-parallel SPMD).

Reference computation (per batch shard):
  x = data @ W.T + b                       # Linear [B,T,I]->[B,T,H]
  s = mean_h(x); a = sigmoid(relu(s@w1.T+b1)@w2.T+b2)   # TA gate [B,T]
  x = x * a[:, :, None]
  LIF over T: v = a*u + x_t; sp = (v>=VTH); u = v*(v<VTH)  # hard reset

Strategy (v2 — single-pass fp32r):
  - Shard B=128 over 8 cores (16 each); W replicated, streamed once.
  - Linear runs as ONE fp32r pass. TRN2's fp32r matmul rounds both
    operands to an 11-bit mantissa but runs at full PE rate (1 row/cyc,
    measured 230ns per 512-row matmul vs fp16's 216ns), halving the
    tensor work vs the old fp16+fp8-DoubleRow scheme. Simulated spike
    error: 574/16.7M flips, rel 0.0117 (gate 2e-2).
  - Tokens are t-major (tok = t*16 + b), so each 512-token PSUM chunk is
    a 32-timestep slab; x stored [128part, hc, t, b] making every drain
    a contiguous [128,512] ACT copy (+ per-partition bias) from PSUM.
  - TA gate: squeeze s = dat.T @ mean_h(W) on TensorE; the tiny MLP is
    two matmuls in [t,b]-partition layout (contraction over T=64 then
    R=4 partitions), sigmoid+bias on ACT. The gate multiplies x as a
    bulk per-(hc,tc) fixup on the GpSimd(Pool) engine, so drains never
    wait for the gate and PSUM never backs up.
  - LIF: one fused custom-DVE op per step (u' = (a*u+x)*((a*u+x)<VTH)),
    registered at build time, writing the membrane trajectory in place
    over x. Chains run per 256-column weight tile (2 hc chunks), so
    each chain starts right after its tile drains; the final chain is
    only 32 steps (~4.5us tail).
  - Spikes: u'==0 exactly iff the neuron fired (hard reset); a bulk
    is_equal on Pool emits fp8 0/1 planes, DMA'd out and transposed on
    the host. All data-dependent FLOPs run on device.
"""

import functools
import numpy as np

ALPHA = 0.3
VTH = 0.3
B, T, I, H = 128, 64, 2048, 2048
NCORES = 8
BL = B // NCORES          # local batch = 16
TOK = BL * T              # 1024 tokens per core, tok = t*BL + b
NTOKC = 2                 # two 512-token chunks = 32 timesteps each
TOKC = TOK // NTOKC       # 512
TCT = TOKC // BL          # 32 timesteps per chunk
IC = I // 128             # 16 contraction chunks
HC = H // 128             # 16 hidden chunks of 128
NTILE = 8                 # weight tiles of 256 h (2 hc chunks each)

_LIF_OP = None


def _register_lif_op():
    """Register the fused LIF step as a custom DVE op (documented
    extension point: per-NEFF uop table, concourse/dve_ops.py)."""
    global _LIF_OP
    if _LIF_OP is not None:
        return _LIF_OP
    from concourse.dve_spec import Spec, Src0, Src1, C0, C1, lower
    from concourse.dve_ops import DveOp, OPS, CUSTOM_DVE_SPECS, _SUB_OPCODE_FOR_NAME
    from concourse.dve_uop import DveOpSpec
    from concourse.bass import dve_ver_for

    name = "LIF_FUSED_STEP"
    for op in OPS:
        if op.name == name:
            _LIF_OP = op
            return op
    v = Src1 * C0 + Src0
    spec = Spec(
        body=v * (v < C1),
        reference=lambda in0, in1, s0, s1, imm2: (
            (in1 * s0 + in0) * ((in1 * s0 + in0) < s1)
        ).astype(np.float32),
    )
    row = 1 + len(OPS)
    _SUB_OPCODE_FOR_NAME[name] = row
    shas = {}
    for ver in ("v3", "v4"):
        try:
            uops = lower(spec, ver=ver)
            shas[ver] = DveOpSpec(name=name, opcode=row, uops=uops, rd1_en=True).sha(ver)
        except Exception:
            pass
    op = DveOp(name, spec, subdim=False, uops_sha=shas)
    OPS.append(op)
    CUSTOM_DVE_SPECS[name] = spec
    _LIF_OP = op
    return op


@functools.cache
def _build():
    import sys
    if "/opt/trn_rl_repo" not in sys.path:
        sys.path.insert(0, "/opt/trn_rl_repo")
    from contextlib import ExitStack
    from concourse import bacc, mybir, tile

    lif_op = _register_lif_op()

    f32 = mybir.dt.float32
    f32r = mybir.dt.float32r
    f8 = mybir.dt.float8e4
    Alu = mybir.AluOpType
    Act = mybir.ActivationFunctionType

    nc = bacc.Bacc("TRN2", target_bir_lowering=False, debug=False)

    dat_d = nc.dram_tensor("dat", [I, TOK], f32r, kind="ExternalInput")
    wt_d = nc.dram_tensor("wt", [I, H], f32r, kind="ExternalInput")
    bias_d = nc.dram_tensor("bias", [128, HC], f32, kind="ExternalInput")
    wbar_d = nc.dram_tensor("wbar", [128, IC], f32r, kind="ExternalInput")
    bbar_d = nc.dram_tensor("bbar", [1, 1], f32, kind="ExternalInput")
    w1t_d = nc.dram_tensor("w1t", [T, 4], f32, kind="ExternalInput")
    w2t_d = nc.dram_tensor("w2t", [4, T], f32, kind="ExternalInput")
    b1c_d = nc.dram_tensor("b1c", [4, 1], f32, kind="ExternalInput")
    b2c_d = nc.dram_tensor("b2c", [T, 1], f32, kind="ExternalInput")
    spk_d = nc.dram_tensor("spk", [NTOKC, 128, TCT, 256], f8, kind="ExternalOutput")

    s_dram = nc.dram_tensor("s_scratch", [NTOKC, TOKC], f32)
    a_dram = nc.dram_tensor("a_scratch", [T, 2 * BL], f32)

    with ExitStack() as ctx:
        tc = ctx.enter_context(tile.TileContext(nc))
        const = ctx.enter_context(tc.tile_pool(name="const", bufs=1))
        wpool = ctx.enter_context(tc.tile_pool(name="wpool", bufs=3))
        psum = ctx.enter_context(tc.tile_pool(name="psum", bufs=6, space="PSUM"))
        psum_s = ctx.enter_context(tc.tile_pool(name="psum_s", bufs=1, space="PSUM"))
        psum_g = ctx.enter_context(tc.tile_pool(name="psum_g", bufs=1, space="PSUM"))

        # ---- persistent tiles ----
        dat_sb = const.tile([128, IC, TOK], f32r, tag="dat")
        # x trajectory: [128, t_local, flat(k,sub,b)=256] per token chunk
        x_sb = [const.tile([128, TCT, 256], f32, tag=f"x{i}", name=f"x{i}")
                for i in range(NTOKC)]
        sp_sb = [const.tile([128, TCT, 256], f8, tag=f"sp{i}", name=f"sp{i}")
                 for i in range(NTOKC)]
        bias_sb = const.tile([128, HC], f32, tag="bias")
        wbar_sb = const.tile([128, IC], f32r, tag="wbar")
        bbar_sb = const.tile([1, 1], f32, tag="bbar")
        w1t_sb = const.tile([T, 4], f32, tag="w1t")
        w2t_sb = const.tile([4, T], f32, tag="w2t")
        b1c_sb = const.tile([4, 1], f32, tag="b1c")
        b2c_sb = const.tile([T, 1], f32, tag="b2c")
        sTT_sb = const.tile([T, BL], f32, tag="sTT")
        h1r_sb = const.tile([4, BL], f32, tag="h1r")
        a_t_sb = const.tile([T, BL], f32, tag="a_t")
        a_rep = const.tile([128, T, 2 * BL], f32, tag="a_rep")
        zeros = const.tile([128, 128], f32, tag="zeros")
        s_sb = [const.tile([1, TOKC], f32, tag=f"s{i}", name=f"s{i}")
                for i in range(NTOKC)]

        nc.vector.memset(zeros, 0.0)

        datv = dat_d.ap().rearrange("(ic p) tok -> p ic tok", p=128)

        # ---- DMA plan: Sync = wsl0, dat-tc0, consts, wsl1..7 (weights are
        # never queued behind spike stores); ACT ring = dat-tc1 + gate
        # bounces; GpSimd SW ring = spike stores only. ----
        wsl = [None] * (NTILE + 1)

        def load_wsl(k):
            w = wpool.tile([128, IC, 256], f32r, tag="wsl", name=f"wsl{k}")
            nc.sync.dma_start(
                out=w, in_=wt_d[:, k * 256:(k + 1) * 256].rearrange(
                    "(ic p) h -> p ic h", p=128)
            )
            wsl[k] = w

        # first tile (=tile 7, processed first) split into 4-ic chunks so
        # the first PSUM group pipelines with DMA arrival
        w7 = wpool.tile([128, IC, 256], f32r, tag="wsl", name="wsl7")
        for icc in range(0, IC, 4):
            nc.sync.dma_start(
                out=w7[:, icc:icc + 4, :],
                in_=wt_d[:, 7 * 256:8 * 256].rearrange(
                    "(ic p) h -> p ic h", p=128)[:, icc:icc + 4, :],
            )
        wsl[7] = w7
        for icc, eng in ((0, nc.sync), (4, nc.gpsimd), (8, nc.scalar), (12, nc.scalar)):
            eng.dma_start(
                out=dat_sb[:, icc:icc + 4, 0:TOKC], in_=datv[:, icc:icc + 4, 0:TOKC]
            )
        nc.scalar.dma_start(out=bias_sb, in_=bias_d.ap())
        nc.scalar.dma_start(out=wbar_sb, in_=wbar_d.ap())
        nc.scalar.dma_start(out=bbar_sb, in_=bbar_d.ap())
        for icc in range(0, IC, 4):
            nc.scalar.dma_start(
                out=dat_sb[:, icc:icc + 4, TOKC:TOK], in_=datv[:, icc:icc + 4, TOKC:TOK]
            )
        nc.scalar.dma_start(out=w1t_sb, in_=w1t_d.ap())
        nc.scalar.dma_start(out=w2t_sb, in_=w2t_d.ap())
        nc.scalar.dma_start(out=b1c_sb, in_=b1c_d.ap())
        nc.scalar.dma_start(out=b2c_sb, in_=b2c_d.ap())
        load_wsl(0)
        load_wsl(1)

        IC_ORD = [x for p in zip(range(0, 8), range(8, 16)) for x in p]

        def emit_squeeze(tci):
            ps = psum_s.tile([1, TOKC], f32, tag="ps_s", name=f"ps_s{tci}")
            for j, ic in enumerate(IC_ORD):
                nc.tensor.matmul(
                    ps, lhsT=wbar_sb[:, ic:ic + 1],
                    rhs=dat_sb[:, ic, tci * TOKC:(tci + 1) * TOKC],
                    start=(j == 0), stop=(j == IC - 1),
                )
            nc.scalar.activation(out=s_sb[tci], in_=ps, func=Act.Identity, bias=bbar_sb)
            nc.scalar.dma_start(out=s_dram.ap()[tci:tci + 1], in_=s_sb[tci])

        def emit_gate():
            for tci in range(NTOKC):
                nc.scalar.dma_start(
                    out=sTT_sb[tci * TCT:(tci + 1) * TCT, :],
                    in_=s_dram.ap()[tci:tci + 1].rearrange(
                        "one (t b) -> one t b", b=BL),
                )
            ps_h1 = psum_g.tile([4, BL], f32, tag="ps_g", name="ps_h1")
            nc.tensor.matmul(ps_h1, lhsT=w1t_sb, rhs=sTT_sb, start=True, stop=True)
            nc.scalar.activation(out=h1r_sb, in_=ps_h1, func=Act.Relu, bias=b1c_sb)
            ps_z = psum_g.tile([T, BL], f32, tag="ps_g", name="ps_z")
            nc.tensor.matmul(ps_z, lhsT=w2t_sb, rhs=h1r_sb, start=True, stop=True)
            nc.scalar.activation(out=a_t_sb, in_=ps_z, func=Act.Sigmoid, bias=b2c_sb)
            nc.scalar.dma_start(out=a_dram.ap()[:, 0:BL], in_=a_t_sb)
            nc.scalar.dma_start(out=a_dram.ap()[:, BL:2 * BL], in_=a_t_sb)
            nc.scalar.dma_start(
                out=a_rep,
                in_=a_dram.ap().unsqueeze(0).to_broadcast((128, T, 2 * BL)),
            )

        def emit_group(k, tci, sub):
            hc = k * 2 + sub
            off = k * 32 + sub * BL
            ps = psum.tile([128, TOKC], f32, tag="ps_mm", name=f"ps_{hc}_{tci}")
            for j, ic in enumerate(IC_ORD):
                nc.tensor.matmul(
                    ps, lhsT=wsl[k][:, ic, sub * 128:sub * 128 + 128],
                    rhs=dat_sb[:, ic, tci * TOKC:(tci + 1) * TOKC],
                    start=(j == 0), stop=(j == IC - 1),
                )
            nc.scalar.activation(
                out=x_sb[tci][:, :, off:off + BL], in_=ps, func=Act.Identity,
                bias=bias_sb[:, hc:hc + 1],
            )

        def emit_fixup(k, tci):
            off = k * 32
            nc.vector.tensor_tensor(
                out=x_sb[tci][:, :, off:off + 32], in0=x_sb[tci][:, :, off:off + 32],
                in1=a_rep[:, tci * TCT:(tci + 1) * TCT, :], op=Alu.mult,
            )

        def emit_chain(span, tci, t0=0, t1=TCT):
            k0, k1 = span
            off, w = k0 * 32, (k1 - k0) * 32
            for tl in range(t0, t1):
                if tci == 0 and tl == 0:
                    prev = zeros[:, :w]
                elif tl == 0:
                    prev = x_sb[0][:, TCT - 1, off:off + w]
                else:
                    prev = x_sb[tci][:, tl - 1, off:off + w]
                nc.vector._custom_dve(
                    lif_op, out=x_sb[tci][:, tl, off:off + w],
                    in0=x_sb[tci][:, tl, off:off + w], in1=prev,
                    s0=ALPHA, s1=VTH,
                )

        def emit_tail_halves():
            k = 7
            for half in range(2):
                t0, t1 = half * 16, (half + 1) * 16
                pss = []
                for sub in range(2):
                    hc = k * 2 + sub
                    ps = psum.tile([128, TOKC], f32, tag="ps_mm",
                                   name=f"ps_t7h{half}s{sub}")
                    for ic in range(IC):
                        nc.tensor.matmul(
                            ps[:, 0:256],
                            lhsT=wsl[8][:, ic, sub * 128:sub * 128 + 128],
                            rhs=dat_sb[:, ic, TOKC + t0 * BL:TOKC + t1 * BL],
                            start=(ic == 0), stop=(ic == IC - 1),
                        )
                    off = k * 32 + sub * BL
                    nc.scalar.activation(
                        out=x_sb[1][:, t0:t1, off:off + BL], in_=ps[:, 0:256],
                        func=Act.Identity, bias=bias_sb[:, hc:hc + 1],
                    )
                nc.vector.tensor_tensor(
                    out=x_sb[1][:, t0:t1, k * 32:k * 32 + 32],
                    in0=x_sb[1][:, t0:t1, k * 32:k * 32 + 32],
                    in1=a_rep[:, TCT + t0:TCT + t1, :], op=Alu.mult,
                )
                emit_chain(SPANS[2], 1, t0, t1)
            for half in range(2):
                t0, t1 = half * 16, (half + 1) * 16
                nc.scalar.activation(
                    out=sp_sb[1][:, t0:t1, k * 32:k * 32 + 32],
                    in_=x_sb[1][:, t0:t1, k * 32:k * 32 + 32], func=Act.Sign,
                )
                nc.scalar.dma_start(
                    out=spk_d.ap()[1:2, :, t0:t1, k * 32:k * 32 + 32],
                    in_=sp_sb[1][:, t0:t1, k * 32:k * 32 + 32],
                )

        def emit_extract(span, tci, eng=None):
            k0, k1 = span
            off, w = k0 * 32, (k1 - k0) * 32
            nc.scalar.activation(
                out=sp_sb[tci][:, :, off:off + w],
                in_=x_sb[tci][:, :, off:off + w], func=Act.Sign,
            )
            (eng or nc.sync).dma_start(
                out=spk_d.ap()[tci:tci + 1, :, :, off:off + w],
                in_=sp_sb[tci][:, :, off:off + w],
            )

        # Schedule: tile 7's tc0 runs FIRST so its LIF chain (the tail
        # span) completes mid-kernel; tiles 0-6 follow; tile 7's tc1 is
        # split into two 16-step halves at the end (short tail chain).
        SPANS = [(0, 4), (4, 7), (7, 8)]
        emit_group(7, 0, 0)
        emit_group(7, 0, 1)
        emit_squeeze(0)
        emit_group(0, 0, 0)
        emit_group(0, 0, 1)
        emit_group(0, 1, 0)
        emit_group(0, 1, 1)
        load_wsl(2)
        emit_group(1, 0, 0)
        emit_group(1, 0, 1)
        emit_squeeze(1)
        emit_group(1, 1, 0)
        emit_group(1, 1, 1)
        load_wsl(3)
        emit_group(2, 0, 0)
        emit_group(2, 0, 1)
        emit_gate()
        emit_fixup(7, 0)
        emit_chain(SPANS[2], 0)
        emit_fixup(0, 0)
        emit_fixup(0, 1)
        emit_fixup(1, 0)
        emit_fixup(1, 1)
        emit_fixup(2, 0)
        emit_group(2, 1, 0)
        emit_group(2, 1, 1)
        emit_fixup(2, 1)
        load_wsl(4)
        for k in range(3, 7):
            emit_group(k, 0, 0)
            emit_group(k, 0, 1)
            emit_fixup(k, 0)
            if k == 3:
                emit_chain(SPANS[0], 0)
            if k == 6:
                emit_chain(SPANS[1], 0)
            emit_group(k, 1, 0)
            emit_group(k, 1, 1)
            emit_fixup(k, 1)
            if k == 3:
                emit_chain(SPANS[0], 1)
            if k == 6:
                emit_chain(SPANS[1], 1)
            if k == 4:
                emit_extract(SPANS[0], 0)
                emit_extract(SPANS[0], 1)
            if k + 2 <= 6:
                load_wsl(k + 2)
            if k == 4:
                # reload tile-7 weights for the tail halves
                w7b = wpool.tile([128, IC, 256], f32r, tag="wsl", name="wsl7b")
                nc.sync.dma_start(
                    out=w7b, in_=wt_d[:, 7 * 256:8 * 256].rearrange(
                        "(ic p) h -> p ic h", p=128))
                wsl[8] = w7b
        emit_tail_halves()
        emit_extract(SPANS[1], 0)
        emit_extract(SPANS[1], 1)
        emit_extract(SPANS[2], 0, eng=nc.scalar)

    nc.compile()
    return nc


def _host_prep(data, W, b, w1, b1, w2, b2):
    data = np.ascontiguousarray(data, dtype=np.float32)
    W = np.ascontiguousarray(W, dtype=np.float32)
    wt = np.ascontiguousarray(W.T)                      # [I, H]
    bias = np.ascontiguousarray(b.reshape(HC, 128).T, dtype=np.float32)
    wbar = W.mean(axis=0, dtype=np.float64).astype(np.float32)
    wbar_t = np.ascontiguousarray(wbar.reshape(IC, 128).T)
    bbar = np.array([[b.mean(dtype=np.float64)]], dtype=np.float32)
    w1t = np.ascontiguousarray(w1.T, dtype=np.float32)  # [T, 4]
    w2t = np.ascontiguousarray(w2.T, dtype=np.float32)  # [4, T]
    b1c = np.ascontiguousarray(b1.reshape(4, 1), dtype=np.float32)
    b2c = np.ascontiguousarray(b2.reshape(T, 1), dtype=np.float32)

    in_maps = []
    for c in range(NCORES):
        dc = data[c * BL:(c + 1) * BL]                  # [BL, T, I]
        dat = np.ascontiguousarray(dc.transpose(2, 1, 0).reshape(I, TOK))
        in_maps.append({
            "dat": dat, "wt": wt, "bias": bias, "wbar": wbar_t, "bbar": bbar,
            "w1t": w1t, "w2t": w2t, "b1c": b1c, "b2c": b2c,
        })
    return in_maps


def _gather(results):
    outs = []
    for c in range(NCORES):
        spk = np.asarray(results[c]["spk"])             # [2, 128, TCT, 256] f8
        raw = spk.view(np.uint8).reshape(NTOKC, 128, TCT, NTILE, 2, BL)
        # Sign(u') in {-1,0,+1}; spike fired iff u'==0 -> byte &0x7f == 0
        sp = ((raw & 0x7F) == 0)
        # [tc, p, tl, k, sub, b] -> [b, tc, tl, k, sub, p]
        outs.append(
            sp.transpose(5, 0, 2, 3, 4, 1).reshape(BL, T, H).astype(np.float32)
        )
    return np.concatenate(outs, axis=0)


def kernel(data, W, b, w1, b1, w2, b2):
    import sys
    if "/opt/trn_rl_repo" not in sys.path:
        sys.path.insert(0, "/opt/trn_rl_repo")
    from concourse.bass_utils import run_bass_kernel_spmd

    nc = _build()
    in_maps = _host_prep(data, W, b, w1, b1, w2, b2)
    res = run_bass_kernel_spmd(nc, in_maps, list(range(NCORES)))
    return _gather(res.results).astype(np.float32)


# revision 17
# speedup vs baseline: 1.0046x; 1.0046x over previous
"""AttLIF Trainium2 kernel (8-core data-parallel SPMD).

Reference computation (per batch shard):
  x = data @ W.T + b                       # Linear [B,T,I]->[B,T,H]
  s = mean_h(x); a = sigmoid(relu(s@w1.T+b1)@w2.T+b2)   # TA gate [B,T]
  x = x * a[:, :, None]
  LIF over T: v = a*u + x_t; sp = (v>=VTH); u = v*(v<VTH)  # hard reset

Strategy (v2 — single-pass fp32r):
  - Shard B=128 over 8 cores (16 each); W replicated, streamed once.
  - Linear runs as ONE fp32r pass. TRN2's fp32r matmul rounds both
    operands to an 11-bit mantissa but runs at full PE rate (1 row/cyc,
    measured 230ns per 512-row matmul vs fp16's 216ns), halving the
    tensor work vs the old fp16+fp8-DoubleRow scheme. Simulated spike
    error: 574/16.7M flips, rel 0.0117 (gate 2e-2).
  - Tokens are t-major (tok = t*16 + b), so each 512-token PSUM chunk is
    a 32-timestep slab; x stored [128part, hc, t, b] making every drain
    a contiguous [128,512] ACT copy (+ per-partition bias) from PSUM.
  - TA gate: squeeze s = dat.T @ mean_h(W) on TensorE; the tiny MLP is
    two matmuls in [t,b]-partition layout (contraction over T=64 then
    R=4 partitions), sigmoid+bias on ACT. The gate multiplies x as a
    bulk per-(hc,tc) fixup on the GpSimd(Pool) engine, so drains never
    wait for the gate and PSUM never backs up.
  - LIF: one fused custom-DVE op per step (u' = (a*u+x)*((a*u+x)<VTH)),
    registered at build time, writing the membrane trajectory in place
    over x. Chains run per 256-column weight tile (2 hc chunks), so
    each chain starts right after its tile drains; the final chain is
    only 32 steps (~4.5us tail).
  - Spikes: u'==0 exactly iff the neuron fired (hard reset); a bulk
    is_equal on Pool emits fp8 0/1 planes, DMA'd out and transposed on
    the host. All data-dependent FLOPs run on device.
"""

import functools
import numpy as np

ALPHA = 0.3
VTH = 0.3
B, T, I, H = 128, 64, 2048, 2048
NCORES = 8
BL = B // NCORES          # local batch = 16
TOK = BL * T              # 1024 tokens per core, tok = t*BL + b
NTOKC = 2                 # two 512-token chunks = 32 timesteps each
TOKC = TOK // NTOKC       # 512
TCT = TOKC // BL          # 32 timesteps per chunk
IC = I // 128             # 16 contraction chunks
HC = H // 128             # 16 hidden chunks of 128
NTILE = 8                 # weight tiles of 256 h (2 hc chunks each)

_LIF_OP = None


def _register_lif_op():
    """Register the fused LIF step as a custom DVE op (documented
    extension point: per-NEFF uop table, concourse/dve_ops.py)."""
    global _LIF_OP
    if _LIF_OP is not None:
        return _LIF_OP
    from concourse.dve_spec import Spec, Src0, Src1, C0, C1, lower
    from concourse.dve_ops import DveOp, OPS, CUSTOM_DVE_SPECS, _SUB_OPCODE_FOR_NAME
    from concourse.dve_uop import DveOpSpec
    from concourse.bass import dve_ver_for

    name = "LIF_FUSED_STEP"
    for op in OPS:
        if op.name == name:
            _LIF_OP = op
            return op
    v = Src1 * C0 + Src0
    spec = Spec(
        body=v * (v < C1),
        reference=lambda in0, in1, s0, s1, imm2: (
            (in1 * s0 + in0) * ((in1 * s0 + in0) < s1)
        ).astype(np.float32),
    )
    row = 1 + len(OPS)
    _SUB_OPCODE_FOR_NAME[name] = row
    shas = {}
    for ver in ("v3", "v4"):
        try:
            uops = lower(spec, ver=ver)
            shas[ver] = DveOpSpec(name=name, opcode=row, uops=uops, rd1_en=True).sha(ver)
        except Exception:
            pass
    op = DveOp(name, spec, subdim=False, uops_sha=shas)
    OPS.append(op)
    CUSTOM_DVE_SPECS[name] = spec
    _LIF_OP = op
    return op


@functools.cache
def _build():
    import sys
    if "/opt/trn_rl_repo" not in sys.path:
        sys.path.insert(0, "/opt/trn_rl_repo")
    from contextlib import ExitStack
    from concourse import bacc, mybir, tile

    lif_op = _register_lif_op()

    f32 = mybir.dt.float32
    f32r = mybir.dt.float32r
    f8 = mybir.dt.float8e4
    Alu = mybir.AluOpType
    Act = mybir.ActivationFunctionType

    nc = bacc.Bacc("TRN2", target_bir_lowering=False, debug=False)

    dat_d = nc.dram_tensor("dat", [I, TOK], f32r, kind="ExternalInput")
    wt_d = nc.dram_tensor("wt", [I, H], f32r, kind="ExternalInput")
    bias_d = nc.dram_tensor("bias", [128, HC], f32, kind="ExternalInput")
    wbar_d = nc.dram_tensor("wbar", [128, IC], f32r, kind="ExternalInput")
    bbar_d = nc.dram_tensor("bbar", [1, 1], f32, kind="ExternalInput")
    w1t_d = nc.dram_tensor("w1t", [T, 4], f32, kind="ExternalInput")
    w2t_d = nc.dram_tensor("w2t", [4, T], f32, kind="ExternalInput")
    b1c_d = nc.dram_tensor("b1c", [4, 1], f32, kind="ExternalInput")
    b2c_d = nc.dram_tensor("b2c", [T, 1], f32, kind="ExternalInput")
    spk_d = nc.dram_tensor("spk", [NTOKC, 128, TCT, 256], f8, kind="ExternalOutput")

    s_dram = nc.dram_tensor("s_scratch", [NTOKC, TOKC], f32)
    a_dram = nc.dram_tensor("a_scratch", [T, 2 * BL], f32)

    with ExitStack() as ctx:
        tc = ctx.enter_context(tile.TileContext(nc))
        const = ctx.enter_context(tc.tile_pool(name="const", bufs=1))
        wpool = ctx.enter_context(tc.tile_pool(name="wpool", bufs=3))
        psum = ctx.enter_context(tc.tile_pool(name="psum", bufs=6, space="PSUM"))
        psum_s = ctx.enter_context(tc.tile_pool(name="psum_s", bufs=1, space="PSUM"))
        psum_g = ctx.enter_context(tc.tile_pool(name="psum_g", bufs=1, space="PSUM"))

        # ---- persistent tiles ----
        dat_sb = const.tile([128, IC, TOK], f32r, tag="dat")
        # x trajectory: [128, t_local, flat(k,sub,b)=256] per token chunk
        x_sb = [const.tile([128, TCT, 256], f32, tag=f"x{i}", name=f"x{i}")
                for i in range(NTOKC)]
        sp_sb = [const.tile([128, TCT, 256], f8, tag=f"sp{i}", name=f"sp{i}")
                 for i in range(NTOKC)]
        bias_sb = const.tile([128, HC], f32, tag="bias")
        wbar_sb = const.tile([128, IC], f32r, tag="wbar")
        bbar_sb = const.tile([1, 1], f32, tag="bbar")
        w1t_sb = const.tile([T, 4], f32, tag="w1t")
        w2t_sb = const.tile([4, T], f32, tag="w2t")
        b1c_sb = const.tile([4, 1], f32, tag="b1c")
        b2c_sb = const.tile([T, 1], f32, tag="b2c")
        sTT_sb = const.tile([T, BL], f32, tag="sTT")
        h1r_sb = const.tile([4, BL], f32, tag="h1r")
        a_t_sb = const.tile([T, BL], f32, tag="a_t")
        a_rep = const.tile([128, T, 2 * BL], f32, tag="a_rep")
        zeros = const.tile([128, 128], f32, tag="zeros")
        s_sb = [const.tile([1, TOKC], f32, tag=f"s{i}", name=f"s{i}")
                for i in range(NTOKC)]

        nc.vector.memset(zeros, 0.0)

        datv = dat_d.ap().rearrange("(ic p) tok -> p ic tok", p=128)

        # ---- DMA plan: Sync = wsl0, dat-tc0, consts, wsl1..7 (weights are
        # never queued behind spike stores); ACT ring = dat-tc1 + gate
        # bounces; GpSimd SW ring = spike stores only. ----
        wsl = [None] * (NTILE + 1)

        def load_wsl(k):
            w = wpool.tile([128, IC, 256], f32r, tag="wsl", name=f"wsl{k}")
            nc.sync.dma_start(
                out=w, in_=wt_d[:, k * 256:(k + 1) * 256].rearrange(
                    "(ic p) h -> p ic h", p=128)
            )
            wsl[k] = w

        # first tile (=tile 7, processed first) split into 4-ic chunks so
        # the first PSUM group pipelines with DMA arrival
        w7 = wpool.tile([128, IC, 256], f32r, tag="wsl", name="wsl7")
        for icc in range(0, IC, 4):
            nc.sync.dma_start(
                out=w7[:, icc:icc + 4, :],
                in_=wt_d[:, 7 * 256:8 * 256].rearrange(
                    "(ic p) h -> p ic h", p=128)[:, icc:icc + 4, :],
            )
        wsl[7] = w7
        for icc in range(0, IC, 4):
            eng = nc.sync if icc < 8 else nc.scalar
            eng.dma_start(
                out=dat_sb[:, icc:icc + 4, 0:TOKC], in_=datv[:, icc:icc + 4, 0:TOKC]
            )
        nc.scalar.dma_start(out=bias_sb, in_=bias_d.ap())
        nc.scalar.dma_start(out=wbar_sb, in_=wbar_d.ap())
        nc.scalar.dma_start(out=bbar_sb, in_=bbar_d.ap())
        for icc in range(0, IC, 4):
            nc.scalar.dma_start(
                out=dat_sb[:, icc:icc + 4, TOKC:TOK], in_=datv[:, icc:icc + 4, TOKC:TOK]
            )
        nc.scalar.dma_start(out=w1t_sb, in_=w1t_d.ap())
        nc.scalar.dma_start(out=w2t_sb, in_=w2t_d.ap())
        nc.scalar.dma_start(out=b1c_sb, in_=b1c_d.ap())
        nc.scalar.dma_start(out=b2c_sb, in_=b2c_d.ap())
        load_wsl(0)
        load_wsl(1)

        IC_ORD = [x for p in zip(range(0, 8), range(8, 16)) for x in p]

        def emit_squeeze(tci):
            ps = psum_s.tile([1, TOKC], f32, tag="ps_s", name=f"ps_s{tci}")
            for j, ic in enumerate(IC_ORD):
                nc.tensor.matmul(
                    ps, lhsT=wbar_sb[:, ic:ic + 1],
                    rhs=dat_sb[:, ic, tci * TOKC:(tci + 1) * TOKC],
                    start=(j == 0), stop=(j == IC - 1),
                )
            nc.scalar.activation(out=s_sb[tci], in_=ps, func=Act.Identity, bias=bbar_sb)
            nc.scalar.dma_start(out=s_dram.ap()[tci:tci + 1], in_=s_sb[tci])

        def emit_gate():
            for tci in range(NTOKC):
                nc.scalar.dma_start(
                    out=sTT_sb[tci * TCT:(tci + 1) * TCT, :],
                    in_=s_dram.ap()[tci:tci + 1].rearrange(
                        "one (t b) -> one t b", b=BL),
                )
            ps_h1 = psum_g.tile([4, BL], f32, tag="ps_g", name="ps_h1")
            nc.tensor.matmul(ps_h1, lhsT=w1t_sb, rhs=sTT_sb, start=True, stop=True)
            nc.scalar.activation(out=h1r_sb, in_=ps_h1, func=Act.Relu, bias=b1c_sb)
            ps_z = psum_g.tile([T, BL], f32, tag="ps_g", name="ps_z")
            nc.tensor.matmul(ps_z, lhsT=w2t_sb, rhs=h1r_sb, start=True, stop=True)
            nc.scalar.activation(out=a_t_sb, in_=ps_z, func=Act.Sigmoid, bias=b2c_sb)
            nc.scalar.dma_start(out=a_dram.ap()[:, 0:BL], in_=a_t_sb)
            nc.scalar.dma_start(out=a_dram.ap()[:, BL:2 * BL], in_=a_t_sb)
            nc.scalar.dma_start(
                out=a_rep,
                in_=a_dram.ap().unsqueeze(0).to_broadcast((128, T, 2 * BL)),
            )

        def emit_group(k, tci, sub):
            hc = k * 2 + sub
            off = k * 32 + sub * BL
            ps = psum.tile([128, TOKC], f32, tag="ps_mm", name=f"ps_{hc}_{tci}")
            for j, ic in enumerate(IC_ORD):
                nc.tensor.matmul(
                    ps, lhsT=wsl[k][:, ic, sub * 128:sub * 128 + 128],
                    rhs=dat_sb[:, ic, tci * TOKC:(tci + 1) * TOKC],
                    start=(j == 0), stop=(j == IC - 1),
                )
            nc.scalar.activation(
                out=x_sb[tci][:, :, off:off + BL], in_=ps, func=Act.Identity,
                bias=bias_sb[:, hc:hc + 1],
            )

        def emit_fixup(k, tci):
            off = k * 32
            nc.vector.tensor_tensor(
                out=x_sb[tci][:, :, off:off + 32], in0=x_sb[tci][:, :, off:off + 32],
                in1=a_rep[:, tci * TCT:(tci + 1) * TCT, :], op=Alu.mult,
            )

        def emit_chain(span, tci, t0=0, t1=TCT):
            k0, k1 = span
            off, w = k0 * 32, (k1 - k0) * 32
            for tl in range(t0, t1):
                if tci == 0 and tl == 0:
                    prev = zeros[:, :w]
                elif tl == 0:
                    prev = x_sb[0][:, TCT - 1, off:off + w]
                else:
                    prev = x_sb[tci][:, tl - 1, off:off + w]
                nc.vector._custom_dve(
                    lif_op, out=x_sb[tci][:, tl, off:off + w],
                    in0=x_sb[tci][:, tl, off:off + w], in1=prev,
                    s0=ALPHA, s1=VTH,
                )

        def emit_tail_halves():
            k = 7
            for half in range(2):
                t0, t1 = half * 16, (half + 1) * 16
                pss = []
                for sub in range(2):
                    hc = k * 2 + sub
                    ps = psum.tile([128, TOKC], f32, tag="ps_mm",
                                   name=f"ps_t7h{half}s{sub}")
                    for ic in range(IC):
                        nc.tensor.matmul(
                            ps[:, 0:256],
                            lhsT=wsl[8][:, ic, sub * 128:sub * 128 + 128],
                            rhs=dat_sb[:, ic, TOKC + t0 * BL:TOKC + t1 * BL],
                            start=(ic == 0), stop=(ic == IC - 1),
                        )
                    off = k * 32 + sub * BL
                    nc.scalar.activation(
                        out=x_sb[1][:, t0:t1, off:off + BL], in_=ps[:, 0:256],
                        func=Act.Identity, bias=bias_sb[:, hc:hc + 1],
                    )
                nc.vector.tensor_tensor(
                    out=x_sb[1][:, t0:t1, k * 32:k * 32 + 32],
                    in0=x_sb[1][:, t0:t1, k * 32:k * 32 + 32],
                    in1=a_rep[:, TCT + t0:TCT + t1, :], op=Alu.mult,
                )
                emit_chain(SPANS[2], 1, t0, t1)
            for half in range(2):
                t0, t1 = half * 16, (half + 1) * 16
                nc.scalar.activation(
                    out=sp_sb[1][:, t0:t1, k * 32:k * 32 + 32],
                    in_=x_sb[1][:, t0:t1, k * 32:k * 32 + 32], func=Act.Sign,
                )
                nc.scalar.dma_start(
                    out=spk_d.ap()[1:2, :, t0:t1, k * 32:k * 32 + 32],
                    in_=sp_sb[1][:, t0:t1, k * 32:k * 32 + 32],
                )

        def emit_extract(span, tci, eng=None):
            k0, k1 = span
            off, w = k0 * 32, (k1 - k0) * 32
            nc.scalar.activation(
                out=sp_sb[tci][:, :, off:off + w],
                in_=x_sb[tci][:, :, off:off + w], func=Act.Sign,
            )
            (eng or nc.sync).dma_start(
                out=spk_d.ap()[tci:tci + 1, :, :, off:off + w],
                in_=sp_sb[tci][:, :, off:off + w],
            )

        # Schedule: tile 7's tc0 runs FIRST so its LIF chain (the tail
        # span) completes mid-kernel; tiles 0-6 follow; tile 7's tc1 is
        # split into two 16-step halves at the end (short tail chain).
        SPANS = [(0, 4), (4, 7), (7, 8)]
        emit_group(7, 0, 0)
        emit_group(7, 0, 1)
        emit_squeeze(0)
        emit_group(0, 0, 0)
        emit_group(0, 0, 1)
        emit_group(0, 1, 0)
        emit_group(0, 1, 1)
        load_wsl(2)
        emit_group(1, 0, 0)
        emit_group(1, 0, 1)
        emit_squeeze(1)
        emit_group(1, 1, 0)
        emit_group(1, 1, 1)
        load_wsl(3)
        emit_group(2, 0, 0)
        emit_group(2, 0, 1)
        emit_gate()
        emit_fixup(7, 0)
        emit_chain(SPANS[2], 0)
        emit_fixup(0, 0)
        emit_fixup(0, 1)
        emit_fixup(1, 0)
        emit_fixup(1, 1)
        emit_fixup(2, 0)
        emit_group(2, 1, 0)
        emit_group(2, 1, 1)
        emit_fixup(2, 1)
        load_wsl(4)
        for k in range(3, 7):
            emit_group(k, 0, 0)
            emit_group(k, 0, 1)
            emit_fixup(k, 0)
            if k == 3:
                emit_chain(SPANS[0], 0)
            if k == 6:
                emit_chain(SPANS[1], 0)
            emit_group(k, 1, 0)
            emit_group(k, 1, 1)
            emit_fixup(k, 1)
            if k == 3:
                emit_chain(SPANS[0], 1)
            if k == 6:
                emit_chain(SPANS[1], 1)
            if k == 4:
                emit_extract(SPANS[0], 0)
                emit_extract(SPANS[0], 1)
            if k + 2 <= 6:
                load_wsl(k + 2)
            if k == 4:
                # reload tile-7 weights for the tail halves
                w7b = wpool.tile([128, IC, 256], f32r, tag="wsl", name="wsl7b")
                nc.sync.dma_start(
                    out=w7b, in_=wt_d[:, 7 * 256:8 * 256].rearrange(
                        "(ic p) h -> p ic h", p=128))
                wsl[8] = w7b
        emit_tail_halves()
        emit_extract(SPANS[1], 0)
        emit_extract(SPANS[1], 1)
        emit_extract(SPANS[2], 0, eng=nc.scalar)

    nc.compile()
    return nc


def _host_prep(data, W, b, w1, b1, w2, b2):
    data = np.ascontiguousarray(data, dtype=np.float32)
    W = np.ascontiguousarray(W, dtype=np.float32)
    wt = np.ascontiguousarray(W.T)                      # [I, H]
    bias = np.ascontiguousarray(b.reshape(HC, 128).T, dtype=np.float32)
    wbar = W.mean(axis=0, dtype=np.float64).astype(np.float32)
    wbar_t = np.ascontiguousarray(wbar.reshape(IC, 128).T)
    bbar = np.array([[b.mean(dtype=np.float64)]], dtype=np.float32)
    w1t = np.ascontiguousarray(w1.T, dtype=np.float32)  # [T, 4]
    w2t = np.ascontiguousarray(w2.T, dtype=np.float32)  # [4, T]
    b1c = np.ascontiguousarray(b1.reshape(4, 1), dtype=np.float32)
    b2c = np.ascontiguousarray(b2.reshape(T, 1), dtype=np.float32)

    in_maps = []
    for c in range(NCORES):
        dc = data[c * BL:(c + 1) * BL]                  # [BL, T, I]
        dat = np.ascontiguousarray(dc.transpose(2, 1, 0).reshape(I, TOK))
        in_maps.append({
            "dat": dat, "wt": wt, "bias": bias, "wbar": wbar_t, "bbar": bbar,
            "w1t": w1t, "w2t": w2t, "b1c": b1c, "b2c": b2c,
        })
    return in_maps


def _gather(results):
    outs = []
    for c in range(NCORES):
        spk = np.asarray(results[c]["spk"])             # [2, 128, TCT, 256] f8
        raw = spk.view(np.uint8).reshape(NTOKC, 128, TCT, NTILE, 2, BL)
        # Sign(u') in {-1,0,+1}; spike fired iff u'==0 -> byte &0x7f == 0
        sp = ((raw & 0x7F) == 0)
        # [tc, p, tl, k, sub, b] -> [b, tc, tl, k, sub, p]
        outs.append(
            sp.transpose(5, 0, 2, 3, 4, 1).reshape(BL, T, H).astype(np.float32)
        )
    return np.concatenate(outs, axis=0)


def kernel(data, W, b, w1, b1, w2, b2):
    import sys
    if "/opt/trn_rl_repo" not in sys.path:
        sys.path.insert(0, "/opt/trn_rl_repo")
    from concourse.bass_utils import run_bass_kernel_spmd

    nc = _build()
    in_maps = _host_prep(data, W, b, w1, b1, w2, b2)
    res = run_bass_kernel_spmd(nc, in_maps, list(range(NCORES)))
    return _gather(res.results).astype(np.float32)


# revision 18
# speedup vs baseline: 1.0524x; 1.0476x over previous
"""AttLIF Trainium2 kernel (8-core data-parallel SPMD).

Reference computation (per batch shard):
  x = data @ W.T + b                       # Linear [B,T,I]->[B,T,H]
  s = mean_h(x); a = sigmoid(relu(s@w1.T+b1)@w2.T+b2)   # TA gate [B,T]
  x = x * a[:, :, None]
  LIF over T: v = a*u + x_t; sp = (v>=VTH); u = v*(v<VTH)  # hard reset

Strategy (v2 — single-pass fp32r):
  - Shard B=128 over 8 cores (16 each); W replicated, streamed once.
  - Linear runs as ONE fp32r pass. TRN2's fp32r matmul rounds both
    operands to an 11-bit mantissa but runs at full PE rate (1 row/cyc,
    measured 230ns per 512-row matmul vs fp16's 216ns), halving the
    tensor work vs the old fp16+fp8-DoubleRow scheme. Simulated spike
    error: 574/16.7M flips, rel 0.0117 (gate 2e-2).
  - Tokens are t-major (tok = t*16 + b), so each 512-token PSUM chunk is
    a 32-timestep slab; x stored [128part, hc, t, b] making every drain
    a contiguous [128,512] ACT copy (+ per-partition bias) from PSUM.
  - TA gate: squeeze s = dat.T @ mean_h(W) on TensorE; the tiny MLP is
    two matmuls in [t,b]-partition layout (contraction over T=64 then
    R=4 partitions), sigmoid+bias on ACT. The gate multiplies x as a
    bulk per-(hc,tc) fixup on the GpSimd(Pool) engine, so drains never
    wait for the gate and PSUM never backs up.
  - LIF: one fused custom-DVE op per step (u' = (a*u+x)*((a*u+x)<VTH)),
    registered at build time, writing the membrane trajectory in place
    over x. Chains run per 256-column weight tile (2 hc chunks), so
    each chain starts right after its tile drains; the final chain is
    only 32 steps (~4.5us tail).
  - Spikes: u'==0 exactly iff the neuron fired (hard reset); a bulk
    is_equal on Pool emits fp8 0/1 planes, DMA'd out and transposed on
    the host. All data-dependent FLOPs run on device.
"""

import functools
import numpy as np

ALPHA = 0.3
VTH = 0.3
B, T, I, H = 128, 64, 2048, 2048
NCORES = 8
BL = B // NCORES          # local batch = 16
TOK = BL * T              # 1024 tokens per core, tok = t*BL + b
NTOKC = 2                 # two 512-token chunks = 32 timesteps each
TOKC = TOK // NTOKC       # 512
TCT = TOKC // BL          # 32 timesteps per chunk
IC = I // 128             # 16 contraction chunks
HC = H // 128             # 16 hidden chunks of 128
NTILE = 8                 # weight tiles of 256 h (2 hc chunks each)

_LIF_OP = None


def _register_lif_op():
    """Register the fused LIF step as a custom DVE op (documented
    extension point: per-NEFF uop table, concourse/dve_ops.py)."""
    global _LIF_OP
    if _LIF_OP is not None:
        return _LIF_OP
    from concourse.dve_spec import Spec, Src0, Src1, C0, C1, lower
    from concourse.dve_ops import DveOp, OPS, CUSTOM_DVE_SPECS, _SUB_OPCODE_FOR_NAME
    from concourse.dve_uop import DveOpSpec
    from concourse.bass import dve_ver_for

    name = "LIF_FUSED_STEP"
    for op in OPS:
        if op.name == name:
            _LIF_OP = op
            return op
    v = Src1 * C0 + Src0
    spec = Spec(
        body=v * (v < C1),
        reference=lambda in0, in1, s0, s1, imm2: (
            (in1 * s0 + in0) * ((in1 * s0 + in0) < s1)
        ).astype(np.float32),
    )
    row = 1 + len(OPS)
    _SUB_OPCODE_FOR_NAME[name] = row
    shas = {}
    for ver in ("v3", "v4"):
        try:
            uops = lower(spec, ver=ver)
            shas[ver] = DveOpSpec(name=name, opcode=row, uops=uops, rd1_en=True).sha(ver)
        except Exception:
            pass
    op = DveOp(name, spec, subdim=False, uops_sha=shas)
    OPS.append(op)
    CUSTOM_DVE_SPECS[name] = spec
    _LIF_OP = op
    return op


@functools.cache
def _build():
    import sys
    if "/opt/trn_rl_repo" not in sys.path:
        sys.path.insert(0, "/opt/trn_rl_repo")
    from contextlib import ExitStack
    from concourse import bacc, mybir, tile

    lif_op = _register_lif_op()

    f32 = mybir.dt.float32
    f32r = mybir.dt.float32r
    f8 = mybir.dt.float8e4
    Alu = mybir.AluOpType
    Act = mybir.ActivationFunctionType

    nc = bacc.Bacc("TRN2", target_bir_lowering=False, debug=False)

    dat_d = nc.dram_tensor("dat", [I, TOK], f32r, kind="ExternalInput")
    wt_d = nc.dram_tensor("wt", [I, H], f32r, kind="ExternalInput")
    bias_d = nc.dram_tensor("bias", [128, HC], f32, kind="ExternalInput")
    wbar_d = nc.dram_tensor("wbar", [128, IC], f32r, kind="ExternalInput")
    bbar_d = nc.dram_tensor("bbar", [1, 1], f32, kind="ExternalInput")
    w1t_d = nc.dram_tensor("w1t", [T, 4], f32, kind="ExternalInput")
    w2t_d = nc.dram_tensor("w2t", [4, T], f32, kind="ExternalInput")
    b1c_d = nc.dram_tensor("b1c", [4, 1], f32, kind="ExternalInput")
    b2c_d = nc.dram_tensor("b2c", [T, 1], f32, kind="ExternalInput")
    spk_d = nc.dram_tensor("spk", [NTOKC, 128, TCT, 256], f8, kind="ExternalOutput")

    s_dram = nc.dram_tensor("s_scratch", [NTOKC, TOKC], f32)
    a_dram = nc.dram_tensor("a_scratch", [T, 2 * BL], f32)

    with ExitStack() as ctx:
        tc = ctx.enter_context(tile.TileContext(nc))
        const = ctx.enter_context(tc.tile_pool(name="const", bufs=1))
        wpool = ctx.enter_context(tc.tile_pool(name="wpool", bufs=3))
        psum = ctx.enter_context(tc.tile_pool(name="psum", bufs=6, space="PSUM"))
        psum_s = ctx.enter_context(tc.tile_pool(name="psum_s", bufs=1, space="PSUM"))
        psum_g = ctx.enter_context(tc.tile_pool(name="psum_g", bufs=1, space="PSUM"))

        # ---- persistent tiles ----
        dat_sb = const.tile([128, IC, TOK], f32r, tag="dat")
        # x trajectory: [128, t_local, flat(k,sub,b)=256] per token chunk
        x_sb = [const.tile([128, TCT, 256], f32, tag=f"x{i}", name=f"x{i}")
                for i in range(NTOKC)]
        sp_sb = [const.tile([128, TCT, 256], f8, tag=f"sp{i}", name=f"sp{i}")
                 for i in range(NTOKC)]
        bias_sb = const.tile([128, HC], f32, tag="bias")
        wbar_sb = const.tile([128, IC], f32r, tag="wbar")
        bbar_sb = const.tile([1, 1], f32, tag="bbar")
        w1t_sb = const.tile([T, 4], f32, tag="w1t")
        w2t_sb = const.tile([4, T], f32, tag="w2t")
        b1c_sb = const.tile([4, 1], f32, tag="b1c")
        b2c_sb = const.tile([T, 1], f32, tag="b2c")
        sTT_sb = const.tile([T, BL], f32, tag="sTT")
        h1r_sb = const.tile([4, BL], f32, tag="h1r")
        a_t_sb = const.tile([T, BL], f32, tag="a_t")
        a_rep = const.tile([128, T, 2 * BL], f32, tag="a_rep")
        zeros = const.tile([128, 128], f32, tag="zeros")
        s_sb = [const.tile([1, TOKC], f32, tag=f"s{i}", name=f"s{i}")
                for i in range(NTOKC)]

        nc.vector.memset(zeros, 0.0)

        datv = dat_d.ap().rearrange("(ic p) tok -> p ic tok", p=128)

        # ---- DMA plan: Sync = wsl0, dat-tc0, consts, wsl1..7 (weights are
        # never queued behind spike stores); ACT ring = dat-tc1 + gate
        # bounces; GpSimd SW ring = spike stores only. ----
        wsl = [None] * (NTILE + 1)

        def load_wsl(k):
            w = wpool.tile([128, IC, 256], f32r, tag="wsl", name=f"wsl{k}")
            nc.sync.dma_start(
                out=w, in_=wt_d[:, k * 256:(k + 1) * 256].rearrange(
                    "(ic p) h -> p ic h", p=128)
            )
            wsl[k] = w

        # first tile (=tile 7, processed first) split into 4-ic chunks so
        # the first PSUM group pipelines with DMA arrival
        w7 = wpool.tile([128, IC, 256], f32r, tag="wsl", name="wsl7")
        for icc in range(0, IC, 4):
            nc.sync.dma_start(
                out=w7[:, icc:icc + 4, :],
                in_=wt_d[:, 7 * 256:8 * 256].rearrange(
                    "(ic p) h -> p ic h", p=128)[:, icc:icc + 4, :],
            )
        wsl[7] = w7
        for icc in range(0, IC, 4):
            eng = nc.sync if icc < 8 else nc.scalar
            eng.dma_start(
                out=dat_sb[:, icc:icc + 4, 0:TOKC], in_=datv[:, icc:icc + 4, 0:TOKC]
            )
        nc.scalar.dma_start(out=bias_sb, in_=bias_d.ap())
        nc.scalar.dma_start(out=wbar_sb, in_=wbar_d.ap())
        nc.scalar.dma_start(out=bbar_sb, in_=bbar_d.ap())
        nc.scalar.dma_start(out=w1t_sb, in_=w1t_d.ap())
        nc.scalar.dma_start(out=w2t_sb, in_=w2t_d.ap())
        nc.scalar.dma_start(out=b1c_sb, in_=b1c_d.ap())
        nc.scalar.dma_start(out=b2c_sb, in_=b2c_d.ap())
        for icc in range(0, IC, 4):
            nc.scalar.dma_start(
                out=dat_sb[:, icc:icc + 4, TOKC:TOK], in_=datv[:, icc:icc + 4, TOKC:TOK]
            )
        load_wsl(0)
        load_wsl(1)

        IC_ORD = [x for p in zip(range(0, 8), range(8, 16)) for x in p]

        def emit_squeeze(tci):
            ps = psum_s.tile([1, TOKC], f32, tag="ps_s", name=f"ps_s{tci}")
            for j, ic in enumerate(IC_ORD):
                nc.tensor.matmul(
                    ps, lhsT=wbar_sb[:, ic:ic + 1],
                    rhs=dat_sb[:, ic, tci * TOKC:(tci + 1) * TOKC],
                    start=(j == 0), stop=(j == IC - 1),
                )
            nc.scalar.activation(out=s_sb[tci], in_=ps, func=Act.Identity, bias=bbar_sb)
            nc.scalar.dma_start(out=s_dram.ap()[tci:tci + 1], in_=s_sb[tci])

        def emit_gate():
            for tci in range(NTOKC):
                nc.scalar.dma_start(
                    out=sTT_sb[tci * TCT:(tci + 1) * TCT, :],
                    in_=s_dram.ap()[tci:tci + 1].rearrange(
                        "one (t b) -> one t b", b=BL),
                )
            ps_h1 = psum_g.tile([4, BL], f32, tag="ps_g", name="ps_h1")
            nc.tensor.matmul(ps_h1, lhsT=w1t_sb, rhs=sTT_sb, start=True, stop=True)
            nc.scalar.activation(out=h1r_sb, in_=ps_h1, func=Act.Relu, bias=b1c_sb)
            ps_z = psum_g.tile([T, BL], f32, tag="ps_g", name="ps_z")
            nc.tensor.matmul(ps_z, lhsT=w2t_sb, rhs=h1r_sb, start=True, stop=True)
            nc.scalar.activation(out=a_t_sb, in_=ps_z, func=Act.Sigmoid, bias=b2c_sb)
            nc.scalar.dma_start(out=a_dram.ap()[:, 0:BL], in_=a_t_sb)
            nc.scalar.dma_start(out=a_dram.ap()[:, BL:2 * BL], in_=a_t_sb)
            nc.scalar.dma_start(
                out=a_rep,
                in_=a_dram.ap().unsqueeze(0).to_broadcast((128, T, 2 * BL)),
            )

        def emit_group(k, tci, sub):
            hc = k * 2 + sub
            off = k * 32 + sub * BL
            ps = psum.tile([128, TOKC], f32, tag="ps_mm", name=f"ps_{hc}_{tci}")
            for j, ic in enumerate(IC_ORD):
                nc.tensor.matmul(
                    ps, lhsT=wsl[k][:, ic, sub * 128:sub * 128 + 128],
                    rhs=dat_sb[:, ic, tci * TOKC:(tci + 1) * TOKC],
                    start=(j == 0), stop=(j == IC - 1),
                )
            nc.scalar.activation(
                out=x_sb[tci][:, :, off:off + BL], in_=ps, func=Act.Identity,
                bias=bias_sb[:, hc:hc + 1],
            )

        def emit_fixup(k, tci):
            off = k * 32
            nc.vector.tensor_tensor(
                out=x_sb[tci][:, :, off:off + 32], in0=x_sb[tci][:, :, off:off + 32],
                in1=a_rep[:, tci * TCT:(tci + 1) * TCT, :], op=Alu.mult,
            )

        def emit_chain(span, tci, t0=0, t1=TCT):
            k0, k1 = span
            off, w = k0 * 32, (k1 - k0) * 32
            for tl in range(t0, t1):
                if tci == 0 and tl == 0:
                    prev = zeros[:, :w]
                elif tl == 0:
                    prev = x_sb[0][:, TCT - 1, off:off + w]
                else:
                    prev = x_sb[tci][:, tl - 1, off:off + w]
                nc.vector._custom_dve(
                    lif_op, out=x_sb[tci][:, tl, off:off + w],
                    in0=x_sb[tci][:, tl, off:off + w], in1=prev,
                    s0=ALPHA, s1=VTH,
                )

        def emit_tail_halves():
            k = 7
            for half in range(2):
                t0, t1 = half * 16, (half + 1) * 16
                pss = []
                for sub in range(2):
                    hc = k * 2 + sub
                    ps = psum.tile([128, TOKC], f32, tag="ps_mm",
                                   name=f"ps_t7h{half}s{sub}")
                    for ic in range(IC):
                        nc.tensor.matmul(
                            ps[:, 0:256],
                            lhsT=wsl[8][:, ic, sub * 128:sub * 128 + 128],
                            rhs=dat_sb[:, ic, TOKC + t0 * BL:TOKC + t1 * BL],
                            start=(ic == 0), stop=(ic == IC - 1),
                        )
                    off = k * 32 + sub * BL
                    nc.scalar.activation(
                        out=x_sb[1][:, t0:t1, off:off + BL], in_=ps[:, 0:256],
                        func=Act.Identity, bias=bias_sb[:, hc:hc + 1],
                    )
                nc.vector.tensor_tensor(
                    out=x_sb[1][:, t0:t1, k * 32:k * 32 + 32],
                    in0=x_sb[1][:, t0:t1, k * 32:k * 32 + 32],
                    in1=a_rep[:, TCT + t0:TCT + t1, :], op=Alu.mult,
                )
                emit_chain(SPANS[2], 1, t0, t1)
            for half in range(2):
                t0, t1 = half * 16, (half + 1) * 16
                nc.scalar.activation(
                    out=sp_sb[1][:, t0:t1, k * 32:k * 32 + 32],
                    in_=x_sb[1][:, t0:t1, k * 32:k * 32 + 32], func=Act.Sign,
                )
                nc.scalar.dma_start(
                    out=spk_d.ap()[1:2, :, t0:t1, k * 32:k * 32 + 32],
                    in_=sp_sb[1][:, t0:t1, k * 32:k * 32 + 32],
                )

        def emit_extract(span, tci, eng=None):
            k0, k1 = span
            off, w = k0 * 32, (k1 - k0) * 32
            nc.scalar.activation(
                out=sp_sb[tci][:, :, off:off + w],
                in_=x_sb[tci][:, :, off:off + w], func=Act.Sign,
            )
            (eng or nc.sync).dma_start(
                out=spk_d.ap()[tci:tci + 1, :, :, off:off + w],
                in_=sp_sb[tci][:, :, off:off + w],
            )

        # Schedule: tile 7's tc0 runs FIRST so its LIF chain (the tail
        # span) completes mid-kernel; tiles 0-6 follow; tile 7's tc1 is
        # split into two 16-step halves at the end (short tail chain).
        SPANS = [(0, 4), (4, 7), (7, 8)]
        emit_group(7, 0, 0)
        emit_group(7, 0, 1)
        emit_squeeze(0)
        emit_group(0, 0, 0)
        emit_group(0, 0, 1)
        emit_squeeze(1)
        emit_group(0, 1, 0)
        emit_group(0, 1, 1)
        load_wsl(2)
        emit_group(1, 0, 0)
        emit_group(1, 0, 1)
        emit_group(1, 1, 0)
        emit_group(1, 1, 1)
        load_wsl(3)
        emit_group(2, 0, 0)
        emit_group(2, 0, 1)
        emit_gate()
        emit_fixup(7, 0)
        emit_chain(SPANS[2], 0)
        emit_fixup(0, 0)
        emit_fixup(0, 1)
        emit_fixup(1, 0)
        emit_fixup(1, 1)
        emit_fixup(2, 0)
        emit_group(2, 1, 0)
        emit_group(2, 1, 1)
        emit_fixup(2, 1)
        load_wsl(4)
        for k in range(3, 7):
            emit_group(k, 0, 0)
            emit_group(k, 0, 1)
            emit_fixup(k, 0)
            if k == 3:
                emit_chain(SPANS[0], 0)
            if k == 6:
                emit_chain(SPANS[1], 0)
            emit_group(k, 1, 0)
            emit_group(k, 1, 1)
            emit_fixup(k, 1)
            if k == 3:
                emit_chain(SPANS[0], 1)
            if k == 6:
                emit_chain(SPANS[1], 1)
            if k == 4:
                emit_extract(SPANS[0], 0)
                emit_extract(SPANS[0], 1)
            if k + 2 <= 6:
                load_wsl(k + 2)
            if k == 4:
                # reload tile-7 weights for the tail halves
                w7b = wpool.tile([128, IC, 256], f32r, tag="wsl", name="wsl7b")
                nc.sync.dma_start(
                    out=w7b, in_=wt_d[:, 7 * 256:8 * 256].rearrange(
                        "(ic p) h -> p ic h", p=128))
                wsl[8] = w7b
        emit_tail_halves()
        emit_extract(SPANS[1], 0)
        emit_extract(SPANS[1], 1)
        emit_extract(SPANS[2], 0, eng=nc.scalar)

    nc.compile()
    return nc


def _host_prep(data, W, b, w1, b1, w2, b2):
    data = np.ascontiguousarray(data, dtype=np.float32)
    W = np.ascontiguousarray(W, dtype=np.float32)
    wt = np.ascontiguousarray(W.T)                      # [I, H]
    bias = np.ascontiguousarray(b.reshape(HC, 128).T, dtype=np.float32)
    wbar = W.mean(axis=0, dtype=np.float64).astype(np.float32)
    wbar_t = np.ascontiguousarray(wbar.reshape(IC, 128).T)
    bbar = np.array([[b.mean(dtype=np.float64)]], dtype=np.float32)
    w1t = np.ascontiguousarray(w1.T, dtype=np.float32)  # [T, 4]
    w2t = np.ascontiguousarray(w2.T, dtype=np.float32)  # [4, T]
    b1c = np.ascontiguousarray(b1.reshape(4, 1), dtype=np.float32)
    b2c = np.ascontiguousarray(b2.reshape(T, 1), dtype=np.float32)

    in_maps = []
    for c in range(NCORES):
        dc = data[c * BL:(c + 1) * BL]                  # [BL, T, I]
        dat = np.ascontiguousarray(dc.transpose(2, 1, 0).reshape(I, TOK))
        in_maps.append({
            "dat": dat, "wt": wt, "bias": bias, "wbar": wbar_t, "bbar": bbar,
            "w1t": w1t, "w2t": w2t, "b1c": b1c, "b2c": b2c,
        })
    return in_maps


def _gather(results):
    outs = []
    for c in range(NCORES):
        spk = np.asarray(results[c]["spk"])             # [2, 128, TCT, 256] f8
        raw = spk.view(np.uint8).reshape(NTOKC, 128, TCT, NTILE, 2, BL)
        # Sign(u') in {-1,0,+1}; spike fired iff u'==0 -> byte &0x7f == 0
        sp = ((raw & 0x7F) == 0)
        # [tc, p, tl, k, sub, b] -> [b, tc, tl, k, sub, p]
        outs.append(
            sp.transpose(5, 0, 2, 3, 4, 1).reshape(BL, T, H).astype(np.float32)
        )
    return np.concatenate(outs, axis=0)


def kernel(data, W, b, w1, b1, w2, b2):
    import sys
    if "/opt/trn_rl_repo" not in sys.path:
        sys.path.insert(0, "/opt/trn_rl_repo")
    from concourse.bass_utils import run_bass_kernel_spmd

    nc = _build()
    in_maps = _host_prep(data, W, b, w1, b1, w2, b2)
    res = run_bass_kernel_spmd(nc, in_maps, list(range(NCORES)))
    return _gather(res.results).astype(np.float32)
